# revision 11
# baseline (speedup 1.0000x reference)
"""MoE 2D router kernel for 8 Trainium2 NeuronCores — v2, transposed-space.

Strategy (pure data parallel, batch-sharded):
  - B=16 batches split across 8 cores (2 per core). Per core, each batch's
    [C=16, H=128, W=128] tensor is viewed as [128, 2048] in SBUF with
    partition p = c*8 + blk (blk = pixel-block of 2048 contiguous pixels),
    so HBM loads are fully contiguous.
  - Everything is computed in TRANSPOSED (pixel-major) space: x and noise
    are PE-transposed (f32, bit-exact) per 128-column group, so the expert
    axis c lands on the free axis with stride 8. There:
      * top-1 / masked top-2 over experts are strided free-axis reduces,
      * per-pixel stats broadcast back over c as stride-0 views (no PE
        selection matmuls, no fp32 LOW/HIGH weight thrash),
      * the softmax denominator is a strided add-reduce (no selsum matmul),
      * G = mask * bcast(exp(m1)/ssum): the reciprocal is a tiny [128,32] op.
  - softplus(t) = Ln(1 + Exp(t)) directly on the combined exp/ln table;
    1/wnoise = Exp(-Ln(wnoise)) on the same table; load = Erf(q) from the
    erf table, batched at the kernel end => 2 activation-table loads total.
  - Outputs are written in transposed layout; the host inverts the
    permutation while unsharding.
"""
import sys

sys.path.insert(0, "/opt/trn_rl_repo")

import numpy as np

B, C, H, W = 16, 16, 128, 128
NCORES = 8
BPC = B // NCORES           # batches per core
HW = H * W                  # 16384 pixels per (batch, channel)
NBLK = 8                    # pixel blocks per batch (HW / 2048)
FB = C * HW // 128          # free size per batch in [128, FB] layout = 2048
NCH = 4                     # 128-col groups per chunk
CHW = 512                   # chunk width
CPB = FB // CHW             # chunks per batch = 4
NCHUNK = BPC * CPB          # chunks per core = 8

_CACHE = {}


def _build():
    import concourse.bacc as bacc
    import concourse.mybir as mybir
    from concourse.tile import TileContext, add_dep_helper

    f32 = mybir.dt.float32
    bf16 = mybir.dt.bfloat16
    AX = mybir.AxisListType
    OP = mybir.AluOpType
    AF = mybir.ActivationFunctionType
    BIGNEG = -1e30

    nc = bacc.Bacc(trn_type="TRN2", target_bir_lowering=False, debug=False,
                   num_devices=NCORES, name="moe_router")

    xd = nc.dram_tensor("x", [BPC, 128, FB], f32, kind="ExternalInput")
    nd = nc.dram_tensor("noise", [BPC, 128, FB], f32, kind="ExternalInput")
    idf_d = nc.dram_tensor("id_f", [128, 128], f32, kind="ExternalInput")
    wgpat_d = nc.dram_tensor("wg_pat", [128, CHW], f32, kind="ExternalInput")
    wnpat_d = nc.dram_tensor("wn_pat", [128, CHW], f32, kind="ExternalInput")
    gd = nc.dram_tensor("g_out", [BPC, CPB, 128, CHW], f32,
                        kind="ExternalOutput")
    ld = nc.dram_tensor("load_out", [BPC, CPB, 128, CHW], f32,
                        kind="ExternalOutput")

    with TileContext(nc) as tc:
        with tc.tile_pool(name="const", bufs=1) as cpool, \
             tc.tile_pool(name="io", bufs=3) as iop, \
             tc.tile_pool(name="work", bufs=2) as wp, \
             tc.tile_pool(name="small", bufs=2) as sp, \
             tc.tile_pool(name="erf", bufs=1) as ep, \
             tc.tile_pool(name="ps_t", bufs=2, space="PSUM") as ps_t:

            consts = [None]

            def _load_consts():
                idf = cpool.tile([128, 128], f32, tag="idf")
                nc.sync.dma_start(out=idf[:, :], in_=idf_d[:, :])
                wgpat = cpool.tile([128, CHW], f32, tag="wgpat")
                nc.sync.dma_start(out=wgpat[:, :], in_=wgpat_d[:, :])
                wnpat = cpool.tile([128, CHW], f32, tag="wnpat")
                nc.sync.dma_start(out=wnpat[:, :], in_=wnpat_d[:, :])
                return idf, wgpat, wnpat

            qts = []
            last_t6 = [None]  # last table-6 ACT instruction

            for chunk in range(NCHUNK):
                bb, ch = divmod(chunk, CPB)
                cs = ch * CHW

                # ---- load inputs ----
                xa = iop.tile([128, CHW], f32, tag="x")
                nc.sync.dma_start(out=xa[:, :], in_=xd[bb, :, cs:cs + CHW])
                na = iop.tile([128, CHW], f32, tag="noise")
                nc.sync.dma_start(out=na[:, :], in_=nd[bb, :, cs:cs + CHW])
                if consts[0] is None:
                    consts[0] = _load_consts()
                idf, wgpat, wnpat = consts[0]

                # ---- PE transposes to pixel-major (bit-exact f32) ----
                xT = ps_t.tile([128, CHW], f32, tag="xT")
                nT = ps_t.tile([128, CHW], f32, tag="nT")
                for g in range(NCH):
                    s = slice(g * 128, (g + 1) * 128)
                    nc.tensor.transpose(xT[:, s], xa[:, s], idf[:, :])
                    nc.tensor.transpose(nT[:, s], na[:, s], idf[:, :])

                # ---- gates in T-space ----
                tv = wp.tile([128, CHW], f32, tag="tv")
                nc.vector.tensor_tensor(tv[:, :], xT[:, :], wnpat[:, :],
                                        op=OP.mult)
                wg = wp.tile([128, CHW], f32, tag="wg")
                nc.vector.tensor_tensor(wg[:, :], xT[:, :], wgpat[:, :],
                                        op=OP.mult)
                eu0 = wp.tile([128, CHW], f32, tag="eu0")
                i = nc.scalar.activation(eu0[:, :], tv[:, :], AF.Exp)
                wn = wp.tile([128, CHW], f32, tag="wn")
                i = nc.scalar.activation(wn[:, :], eu0[:, :], AF.Ln, bias=1.0)
                lw = wp.tile([128, CHW], f32, tag="lw")
                i = nc.scalar.activation(lw[:, :], wn[:, :], AF.Ln)
                rw = wp.tile([128, CHW], f32, tag="rw")
                i = nc.scalar.activation(rw[:, :], lw[:, :], AF.Exp, scale=-1.0)
                nw = wp.tile([128, CHW], f32, tag="nw")
                nc.vector.tensor_tensor(nw[:, :], nT[:, :], wn[:, :], op=OP.mult)
                hl = wp.tile([128, CHW], f32, tag="hl")
                nc.gpsimd.tensor_tensor(hl[:, :], wg[:, :], nw[:, :], op=OP.add)
                et = wp.tile([128, CHW], bf16, tag="et")
                i = nc.scalar.activation(et[:, :], hl[:, :], AF.Exp)
                last_t6[0] = i

                # ---- expert-axis stats (free-axis strided reduces) ----
                vh = hl[:, :].rearrange("p (g c k) -> p g k c", g=NCH, c=C)
                m1c = sp.tile([128, 32], f32, tag="m1c")
                nc.vector.tensor_reduce(m1c[:, :], vh, axis=AX.X, op=OP.max)
                m1b = (m1c[:, :].rearrange("p (g k) -> p g k", g=NCH)
                       .unsqueeze(2).broadcast_to([128, NCH, C, NBLK]))
                mk = wp.tile([128, CHW], bf16, tag="mk")
                nc.vector.tensor_tensor(mk[:, :], hl[:, :], m1b, op=OP.is_equal)
                md = wp.tile([128, CHW], f32, tag="md")
                nc.vector.scalar_tensor_tensor(md[:, :], mk[:, :], BIGNEG,
                                               hl[:, :], op0=OP.mult, op1=OP.add)
                vm = md[:, :].rearrange("p (g c k) -> p g k c", g=NCH, c=C)
                m2c = sp.tile([128, 32], f32, tag="m2c")
                nc.vector.tensor_reduce(m2c[:, :], vm, axis=AX.X, op=OP.max)
                s2c = sp.tile([128, 32], f32, tag="s2c")
                nc.vector.tensor_tensor(s2c[:, :], m2c[:, :], m1c[:, :],
                                        op=OP.subtract)
                ve = et[:, :].rearrange("p (g c k) -> p g k c", g=NCH, c=C)
                ssc = sp.tile([128, 32], f32, tag="ssc")
                nc.vector.tensor_reduce(ssc[:, :], ve, axis=AX.X, op=OP.add)

                # ---- G = mask * bcast(exp(m1)/ssum) ----
                em = sp.tile([128, 32], f32, tag="em")
                i = nc.scalar.activation(em[:, :], m1c[:, :], AF.Exp)
                last_t6[0] = i
                src = sp.tile([128, 32], f32, tag="src")
                nc.vector.reciprocal(src[:, :], ssc[:, :])
                g1c = sp.tile([128, 32], f32, tag="g1c")
                nc.vector.tensor_tensor(g1c[:, :], em[:, :], src[:, :],
                                        op=OP.mult)
                g1b = (g1c[:, :].rearrange("p (g k) -> p g k", g=NCH)
                       .unsqueeze(2).broadcast_to([128, NCH, C, NBLK]))
                gt = iop.tile([128, CHW], f32, tag="g")
                nc.vector.tensor_tensor(gt[:, :], mk[:, :], g1b, op=OP.mult)
                nc.sync.dma_start(out=gd[bb, ch, :, :], in_=gt[:, :])

                # ---- erf argument: q = (wg - m1 - mk*(m2-m1)) / wnoise ----
                s2b = (s2c[:, :].rearrange("p (g k) -> p g k", g=NCH)
                       .unsqueeze(2).broadcast_to([128, NCH, C, NBLK]))
                d1 = wp.tile([128, CHW], f32, tag="d1")
                nc.vector.tensor_tensor(d1[:, :], wg[:, :], m1b, op=OP.subtract)
                t1 = wp.tile([128, CHW], f32, tag="t1")
                nc.gpsimd.tensor_tensor(t1[:, :], mk[:, :], s2b, op=OP.mult)
                numer = wp.tile([128, CHW], f32, tag="numer")
                nc.gpsimd.tensor_tensor(numer[:, :], d1[:, :], t1[:, :],
                                        op=OP.subtract)
                qt = ep.tile([128, CHW], f32, tag=f"q{chunk}")
                nc.vector.tensor_tensor(qt[:, :], numer[:, :], rw[:, :],
                                        op=OP.mult)
                qts.append((bb, ch, qt))

            # ---- erf tail for all chunks (one erf-table load) ----
            for bb, ch, qt in qts:
                lt = iop.tile([128, CHW], f32, tag=f"load{ch % 2}")
                i = nc.scalar.activation(lt[:, :], qt[:, :], AF.Erf)
                # force every erf after ALL exp/ln ACT ops so the erf table
                # is loaded exactly once (the tile scheduler would otherwise
                # interleave erf with exp/ln and thrash table loads)
                add_dep_helper(last_t6[0].ins, i.ins, sync=True,
                               reason="erf after all exp/ln act ops")
                nc.sync.dma_start(out=ld[bb, ch, :, :], in_=lt[:, :])

    nc.compile()
    _fix_act_tables(nc, mybir)
    return nc


def _fix_act_tables(nc, mybir):
    """Retarget Exp/Ln activation-table loads to the combined exp+ln table
    and Erf loads to the erf-bearing table, then drop redundant reloads."""
    from concourse.hw_specs import get_activation_tables
    AFT = mybir.ActivationFunctionType
    tabs = list(get_activation_tables(nc.m.arch).items())
    targets = []
    for i, (_, fs) in enumerate(tabs):
        if AFT.Exp in fs and AFT.Ln in fs:
            targets.append((i, fs))
    for i, (_, fs) in enumerate(tabs):
        if AFT.Erf in fs:
            targets.append((i, fs))
    for blk in nc.m.functions[0].blocks:
        insts = blk.instructions
        loads = [(idx, inst) for idx, inst in enumerate(insts)
                 if isinstance(inst, mybir.InstLoadActFuncSet)]
        for li, (idx, load) in enumerate(loads):
            end = loads[li + 1][0] if li + 1 < len(loads) else len(insts)
            funcs = {i2.func for i2 in insts[idx + 1:end]
                     if isinstance(i2, mybir.InstActivation)}
            if not funcs:
                continue
            for tid, fs in targets:
                if funcs.issubset(fs):
                    load.act_func_set_id = tid
                    break
        cur = None
        to_remove = []
        for inst in insts:
            if isinstance(inst, mybir.InstLoadActFuncSet):
                if inst.act_func_set_id == cur and not inst.has_wait():
                    to_remove.append(inst)
                else:
                    cur = inst.act_func_set_id
            elif isinstance(inst, mybir.InstActivation):
                assert inst.func in tabs[cur][1], (inst.func, cur)
        for inst in to_remove:
            insts.remove(inst)


def make_in_maps(x, noise, wg_param, wnoise_param):
    identity = np.eye(128, dtype=np.float32)
    # free-axis patterns in T-space: f = g*128 + c*8 + blk -> param[c]
    wgv = np.ascontiguousarray(wg_param, dtype=np.float32).reshape(C)
    wnv = np.ascontiguousarray(wnoise_param, dtype=np.float32).reshape(C)
    wg_pat = np.ascontiguousarray(
        np.broadcast_to(np.tile(np.repeat(wgv, NBLK), NCH), (128, CHW)))
    wn_pat = np.ascontiguousarray(
        np.broadcast_to(np.tile(np.repeat(wnv, NBLK), NCH), (128, CHW)))
    x = np.ascontiguousarray(x, dtype=np.float32)
    noise = np.ascontiguousarray(noise, dtype=np.float32)
    in_maps = []
    for i in range(NCORES):
        xs = x[i * BPC:(i + 1) * BPC].reshape(BPC, 128, FB)
        ns = noise[i * BPC:(i + 1) * BPC].reshape(BPC, 128, FB)
        in_maps.append({"x": xs, "noise": ns, "id_f": identity,
                        "wg_pat": wg_pat, "wn_pat": wn_pat})
    return in_maps


def _decode_T(arr):
    """[BPC, CPB, 128, CHW] T-layout -> [BPC, C, H, W] standard layout.

    arr[bb, ch, pT, g*128 + c*8 + blk] = out[bb, c, blk*2048 + ch*512
                                             + g*128 + pT]
    """
    a = np.asarray(arr, dtype=np.float32).reshape(BPC, CPB, 128, NCH, C, NBLK)
    a = a.transpose(0, 4, 5, 1, 3, 2)  # [bb, c, blk, ch, g, pT]
    return a.reshape(BPC, C, H, W)


def kernel(x, noise, wg_param, wnoise_param):
    from concourse.bass_utils import run_bass_kernel_spmd

    if "nc" not in _CACHE:
        _CACHE["nc"] = _build()
    nc = _CACHE["nc"]
    in_maps = make_in_maps(x, noise, wg_param, wnoise_param)
    res = run_bass_kernel_spmd(nc, in_maps, list(range(NCORES)))
    G = np.empty((B, C, H, W), dtype=np.float32)
    L = np.empty((B, C, H, W), dtype=np.float32)
    for i in range(NCORES):
        G[i * BPC:(i + 1) * BPC] = _decode_T(res.results[i]["g_out"])
        L[i * BPC:(i + 1) * BPC] = _decode_T(res.results[i]["load_out"])
    return G, L
